# revision 9
# baseline (speedup 1.0000x reference)
"""Trainium2 Bass kernel for nn_GaussianBlurDM: per-sample gaussian blur (dense
matrix sandwich on TensorE), 3x3 conv -> relu -> 3x3 conv, MSE loss vs input.
Data-parallel over 8 NeuronCores (4 samples each); scalar loss reduced on host.

Hardcoded problem: B=32, C=3, H=W=256, HID=32, KS=29, NT=1000.
"""
import sys, os
for p in ('/opt/trn_rl_repo', '/root/.axon_site/_ro/trn_rl_repo'):
    if p not in sys.path and os.path.isdir(p):
        sys.path.insert(0, p)

import numpy as np
import ml_dtypes

bf16 = ml_dtypes.bfloat16

B, C, H, W = 32, 3, 256, 256
HID, KS, NT = 32, 29, 1000
NCORES = 8
B4 = B // NCORES          # samples per core
NS = 127                  # conv strips (stride 2, height-4 windows)
PW = 258                  # w-padded row length
ZPITCH = C * PW           # 774
R1PITCH = NS * 256        # 32512

_cached = {}


def _blur_matrix(sigma):
    half = (KS - 1) * 0.5
    xg = np.linspace(-half, half, KS)
    g = np.exp(-0.5 * (xg / sigma) ** 2)
    g = (g / g.sum()).astype(np.float64)
    pad = KS // 2
    A = np.zeros((H, H + 2 * pad), np.float64)
    for i in range(H):
        A[i, i:i + KS] = g
    P = np.zeros((H + 2 * pad, H), np.float64)
    for m in range(H + 2 * pad):
        j = m - pad
        if j < 0:
            j = -j
        elif j >= H:
            j = 2 * (H - 1) - j
        P[m, j] = 1.0
    return (A @ P).astype(np.float32)


def _host_prep(x, t, W1, b1, tw, W2, b2, sigma_schedule, shard):
    xs = np.asarray(x)[shard]
    ts = np.asarray(t)[shard]
    sig = np.asarray(sigma_schedule)[ts]
    tn = ts.astype(np.float32) / NT
    W1 = np.asarray(W1); b1 = np.asarray(b1); tw = np.asarray(tw)
    W2 = np.asarray(W2); b2 = np.asarray(b2)

    Mt = np.stack([_blur_matrix(s).T for s in sig]).astype(bf16)   # [B4,256,256]

    # conv1 stationary: rows (dx,hc,c) 0..53, cols (hj,o)=hj*32+o
    W1L = np.zeros((64, 128), np.float32)
    for dx in range(3):
        for hc in range(6):
            for c in range(C):
                row = dx * 18 + hc * 3 + c
                for hj in range(4):
                    ky = hc - hj
                    if 0 <= ky <= 2:
                        W1L[row, hj * 32:(hj + 1) * 32] = W1[:, c, ky, dx]
    W1L = np.broadcast_to(W1L, (B4, 64, 128)).astype(bf16)

    # conv1 bias per psum partition (hj,o): b1[o] + tn*tw[o]  -> [128, B4]
    BIAS = np.zeros((128, B4), np.float32)
    for b in range(B4):
        BIAS[:, b] = np.tile(b1 + tn[b] * tw, 4)

    # conv2 stationary variants [var(3) x dx(3)] each [128, 32]
    L2 = np.zeros((3, 3, 128, 32), np.float32)
    for dxi in range(3):
        for op in range(3):
            for jp in (1, 2):
                m = op * 2 + (jp - 1)
                for dy in (-1, 0, 1):
                    hj = jp + dy
                    L2[:, dxi, hj * 32:hj * 32 + HID, m] = W2[op, :, dy + 1, dxi]
        for op in range(3):           # var1: s=0, extra h=0 outputs at cols 6..8
            for dy in (0, 1):
                L2[1, dxi, dy * 32:dy * 32 + HID, 6 + op] = W2[op, :, dy + 1, dxi]
        for op in range(3):           # var2: s=126, extra h=255 outputs
            for dy in (-1, 0):
                hj = 3 + dy
                L2[2, dxi, hj * 32:hj * 32 + HID, 6 + op] = W2[op, :, dy + 1, dxi]
    L2 = L2.reshape(9, 128, 32).astype(bf16)

    # x_loss [B4, 128, 32, 256]: rows 32*sub + m hold x - b2
    xl = np.zeros((B4, 128, 32, 256), np.float32)
    for S in range(32):
        for sub in range(4):
            s = 4 * S + sub
            if s >= NS:
                continue
            for op in range(3):
                for jp in (1, 2):
                    m = op * 2 + (jp - 1)
                    xl[:, 32 * sub + m, S, :] = xs[:, op, 2 * s + jp, :] - b2[op]
            if s == 0:
                for op in range(3):
                    xl[:, 6 + op, S, :] = xs[:, op, 0, :] - b2[op]
            if s == 126:
                for op in range(3):
                    xl[:, 64 + 6 + op, S, :] = xs[:, op, 255, :] - b2[op]
    XL = xl.astype(bf16)

    X = xs.astype(bf16)
    return {"X": X, "MT": Mt, "W1L": W1L, "BIAS": BIAS,
            "L2": L2, "XL": XL}


def _build_module():
    import concourse.bacc as bacc
    import concourse.tile as tile
    from concourse import mybir
    from concourse.ap import AP

    BF = mybir.dt.bfloat16
    F32 = mybir.dt.float32
    RELU = mybir.ActivationFunctionType.Relu
    SQUARE = mybir.ActivationFunctionType.Square

    nc = bacc.Bacc("TRN2", target_bir_lowering=False, debug=False,
                   num_devices=NCORES)
    dX = nc.dram_tensor("X", [B4, C, H, W], BF, kind="ExternalInput").ap()
    dMT = nc.dram_tensor("MT", [B4, 256, 256], BF, kind="ExternalInput").ap()
    dW1L = nc.dram_tensor("W1L", [B4, 64, 128], BF, kind="ExternalInput").ap()
    dBIAS = nc.dram_tensor("BIAS", [128, B4], F32, kind="ExternalInput").ap()
    dL2 = nc.dram_tensor("L2", [9, 128, 32], BF, kind="ExternalInput").ap()
    dXL = nc.dram_tensor("XL", [B4, 128, 32, 256], BF, kind="ExternalInput").ap()
    dACC = nc.dram_tensor("ACC", [128, 32], F32, kind="ExternalOutput").ap()
    # internal DRAM staging for the blurred image, h- and w-padded:
    # layout [h_pad(258), c(3), w_pad(258)]
    dZ = nc.dram_tensor("ZSTAGE", [258, C, PW], BF).ap()

    with tile.TileContext(nc) as tc:
        from contextlib import ExitStack
        ctx = ExitStack()
        persist = ctx.enter_context(tc.tile_pool(name="persist", bufs=1))
        io = ctx.enter_context(tc.tile_pool(name="io", bufs=2))
        hpool = ctx.enter_context(tc.tile_pool(name="hpool", bufs=2))
        dpool = ctx.enter_context(tc.tile_pool(name="dpool", bufs=3))
        psA = ctx.enter_context(tc.tile_pool(name="psA", bufs=2, space="PSUM"))
        ps1 = ctx.enter_context(tc.tile_pool(name="ps1", bufs=2, space="PSUM"))
        ps2 = ctx.enter_context(tc.tile_pool(name="ps2", bufs=1, space="PSUM"))

        # persistent tiles
        r1 = persist.tile([64, R1PITCH], BF, tag="r1")
        zn = [persist.tile([128, ZPITCH], BF, tag=f"zn{k}", name=f"zn{k}") for k in range(2)]
        at = [persist.tile([128, C * 256], BF, tag=f"at{k}", name=f"at{k}") for k in range(2)]
        acc = persist.tile([128, 32], F32, tag="acc")
        l2 = persist.tile([128, 9 * 32], BF, tag="l2")
        w1l = persist.tile([64, B4 * 128], BF, tag="w1l")
        bias = persist.tile([128, B4], F32, tag="bias")
        zrow = persist.tile([2, ZPITCH], BF, tag="zrow")

        # one-time init
        for k in range(2):
            nc.gpsimd.memset(zn[k][:], 0.0)
        nc.gpsimd.memset(acc[:], 0.0)
        nc.gpsimd.memset(zrow[:], 0.0)
        # zero the h-pad rows (0 and 257) of the DRAM z staging buffer
        nc.sync.dma_start(AP(dZ.tensor, 0, [[257 * ZPITCH, 2], [1, ZPITCH]]),
                          zrow[:])
        nc.sync.dma_start(l2[:], AP(dL2.tensor, 0,
                                    [[32, 128], [128 * 32, 9], [1, 32]]))
        nc.sync.dma_start(w1l[:], AP(dW1L.tensor, 0,
                                     [[128, 64], [64 * 128, B4], [1, 128]]))
        nc.sync.dma_start(bias[:], dBIAS[:])

        for b in range(B4):
            # ---------------- load inputs for sample b ----------------
            mt = [io.tile([128, 256], BF, tag=f"mt{k}", name=f"mt{k}") for k in range(2)]
            for k in range(2):
                nc.sync.dma_start(mt[k][:], dMT[b, 128 * k:128 * (k + 1), :])
            xc = [[io.tile([128, 256], BF, tag=f"xc{c}{k}", name=f"xc{c}{k}") for k in range(2)]
                  for c in range(C)]
            for c in range(C):
                for k in range(2):
                    nc.sync.dma_start(xc[c][k][:],
                                      dX[b, c, 128 * k:128 * (k + 1), :])

            # ---------------- blur pass A: AT = X^T @ Mt ----------------
            for c in range(C):
                for wk in range(2):
                    pa = psA.tile([128, 256], F32, tag="pab")
                    for hk in range(2):
                        nc.tensor.matmul(pa[:],
                                         xc[c][hk][:, 128 * wk:128 * (wk + 1)],
                                         mt[hk][:], start=(hk == 0), stop=(hk == 1))
                    nc.vector.tensor_copy(at[wk][:, 256 * c:256 * (c + 1)], pa[:])

            # ---------------- blur pass B: z chunks (h' in [0,128),[128,256)) ----
            for c in range(C):
                for mk in range(2):
                    pb = psA.tile([128, 256], F32, tag="pab")
                    for wk in range(2):
                        nc.tensor.matmul(pb[:],
                                         at[wk][:, 256 * c + 128 * mk:
                                                256 * c + 128 * mk + 128],
                                         mt[wk][:], start=(wk == 0), stop=(wk == 1))
                    nc.vector.tensor_copy(zn[mk][:, PW * c + 1:PW * c + 257], pb[:])

            # stage z to DRAM: zn[k] [h-part, (c,w)] -> dZ rows 1+128k..128+128k
            for k in range(2):
                nc.sync.dma_start(
                    AP(dZ.tensor, (1 + 128 * k) * ZPITCH, [[ZPITCH, 128], [1, ZPITCH]]),
                    zn[k][:])

            # ---------------- R1 gather: 3 bulk DMAs from DRAM ----------------
            # R1[(dxi,hc,c), (s,w)] = z[c, 2s-1+hc, w+dxi-1] (padded indexing)
            for dxi in range(3):
                in_ap = AP(dZ.tensor, dxi, [[258, 18], [2 * ZPITCH, NS], [1, 256]])
                out_ap = AP(r1[:].tensor, r1[:].offset + (dxi * 18) * R1PITCH,
                            [[R1PITCH, 18], [256, NS], [1, 256]])
                nc.sync.dma_start(out_ap, in_ap)

            # ---------------- banded conv1 -> H -> conv2 -> loss ----------------
            for band in range(4):
                sband = 32 * band
                hbuf = hpool.tile([128, 32 * PW], BF, tag="H")
                # zero the w-pad columns (cheap: 2x 32 elems/partition)
                for colo in (0, 257):
                    zp = AP(hbuf[:].tensor, hbuf[:].offset + colo,
                            [[32 * PW, 128], [PW, 32], [1, 1]])
                    nc.gpsimd.memset(zp, 0.0)

                # conv1: quads of strips share one 4-bank psum tile
                for q in range(8):
                    sq = sband + 4 * q
                    if sq >= NS:
                        break
                    po = ps1.tile([128, 1024], F32, tag="po")
                    nq = min(4, NS - sq)
                    for i in range(nq):
                        s = sq + i
                        nc.tensor.matmul(po[:, 256 * i:256 * (i + 1)],
                                         w1l[0:54, 128 * b:128 * (b + 1)],
                                         r1[0:54, 256 * s:256 * (s + 1)],
                                         start=True, stop=True)
                    # relu+bias evac into H (w-offset 1 per strip segment), on ACT
                    lo = (sq - sband) * PW
                    out_ap = AP(hbuf[:].tensor, hbuf[:].offset + lo + 1,
                                [[32 * PW, 128], [PW, nq], [1, 256]])
                    in_ap = AP(po[:].tensor, po[:].offset,
                               [[1024, 128], [256, nq], [1, 256]])
                    nc.scalar.activation(out_ap, in_ap, RELU,
                                         bias=bias[:, b:b + 1])

                # conv2 + loss per S-quad (4 S-groups = 16 strips)
                xlb = dpool.tile([128, 2048], BF, tag="xl")
                nc.sync.dma_start(
                    xlb[:], AP(dXL.tensor, dXL[b].offset + band * 8 * 256,
                               [[32 * 256, 128], [256, 8], [1, 256]]))
                for half in range(2):
                    p2 = ps2.tile([128, 1024], F32, tag="p2")
                    for Sk in range(4):
                        S = 8 * band + 4 * half + Sk
                        for sub in range(4):
                            s = 4 * S + sub
                            if s >= NS:
                                nc.vector.memset(
                                    p2[32 * sub:32 * (sub + 1),
                                       256 * Sk:256 * (Sk + 1)], 0.0)
                                continue
                            var = 1 if s == 0 else (2 if s == 126 else 0)
                            sl = (s - sband) * PW
                            for dxi in range(3):
                                nc.tensor.matmul(
                                    p2[32 * sub:32 * (sub + 1),
                                       256 * Sk:256 * (Sk + 1)],
                                    l2[:, (var * 3 + dxi) * 32:
                                          (var * 3 + dxi + 1) * 32],
                                    hbuf[:, sl + dxi:sl + dxi + 256],
                                    start=(dxi == 0), stop=(dxi == 2),
                                    tile_position=(0, 32 * sub))
                    # d = psum - x ; acc += d^2
                    dsb = dpool.tile([128, 1024], BF, tag="d")
                    nc.vector.tensor_sub(dsb[:], p2[:],
                                         xlb[:, 1024 * half:1024 * (half + 1)])
                    jsb = dpool.tile([128, 1024], BF, tag="j")
                    col = b * 8 + band * 2 + half
                    nc.scalar.activation(jsb[:], dsb[:], SQUARE,
                                         accum_out=acc[:, col:col + 1])

        nc.sync.dma_start(dACC[:], acc[:])
        ctx.close()

    nc.compile()
    return nc


def kernel(x, t, W1, b1, tw, W2, b2, sigma_schedule):
    from concourse.bass_utils import run_bass_kernel_spmd

    if "nc" not in _cached:
        _cached["nc"] = _build_module()
    nc = _cached["nc"]

    in_maps = []
    for core in range(NCORES):
        shard = list(range(core * B4, (core + 1) * B4))
        in_maps.append(_host_prep(x, t, W1, b1, tw, W2, b2, sigma_schedule,
                                  shard))
    res = run_bass_kernel_spmd(nc, in_maps, list(range(NCORES)))
    total = 0.0
    for r in res.results:
        total += float(r["ACC"].astype(np.float64).sum())
    out = np.float32(total / (B * C * H * W))
    return np.asarray(out)


if __name__ == "__main__":
    sys.path.insert(0, os.path.dirname(os.path.abspath(__file__)))
    import reference
    inputs = {k: np.asarray(v) for k, v in reference.setup_inputs().items()}
    expected = float(reference.reference(**inputs))
    got = kernel(**inputs)
    rel = abs(float(got) - expected) / abs(expected)
    print("expected", expected, "got", float(got), "rel", rel)


# revision 12
# speedup vs baseline: 1.1221x; 1.1221x over previous
"""Trainium2 Bass kernel for nn_GaussianBlurDM: per-sample gaussian blur (dense
matrix sandwich on TensorE), 3x3 conv -> relu -> 3x3 conv, MSE loss vs input.
Data-parallel over 8 NeuronCores (4 samples each); scalar loss reduced on host.

Hardcoded problem: B=32, C=3, H=W=256, HID=32, KS=29, NT=1000.
"""
import sys, os
for p in ('/opt/trn_rl_repo', '/root/.axon_site/_ro/trn_rl_repo'):
    if p not in sys.path and os.path.isdir(p):
        sys.path.insert(0, p)

import numpy as np
import ml_dtypes

bf16 = ml_dtypes.bfloat16

B, C, H, W = 32, 3, 256, 256
HID, KS, NT = 32, 29, 1000
NCORES = 8
B4 = B // NCORES          # samples per core
NS = 127                  # conv strips (stride 2, height-4 windows)
PW = 258                  # w-padded row length
ZPITCH = C * PW           # 774
R1PITCH = NS * 256        # 32512

_cached = {}


def _blur_matrix(sigma):
    half = (KS - 1) * 0.5
    xg = np.linspace(-half, half, KS)
    g = np.exp(-0.5 * (xg / sigma) ** 2)
    g = (g / g.sum()).astype(np.float64)
    pad = KS // 2
    A = np.zeros((H, H + 2 * pad), np.float64)
    for i in range(H):
        A[i, i:i + KS] = g
    P = np.zeros((H + 2 * pad, H), np.float64)
    for m in range(H + 2 * pad):
        j = m - pad
        if j < 0:
            j = -j
        elif j >= H:
            j = 2 * (H - 1) - j
        P[m, j] = 1.0
    return (A @ P).astype(np.float32)


def _host_prep(x, t, W1, b1, tw, W2, b2, sigma_schedule, shard):
    xs = np.asarray(x)[shard]
    ts = np.asarray(t)[shard]
    sig = np.asarray(sigma_schedule)[ts]
    tn = ts.astype(np.float32) / NT
    W1 = np.asarray(W1); b1 = np.asarray(b1); tw = np.asarray(tw)
    W2 = np.asarray(W2); b2 = np.asarray(b2)

    Mt = np.stack([_blur_matrix(s).T for s in sig]).astype(bf16)   # [B4,256,256]

    # conv1 stationary: rows (dx,hc,c) 0..53, cols (hj,o)=hj*32+o
    W1L = np.zeros((64, 128), np.float32)
    for dx in range(3):
        for hc in range(6):
            for c in range(C):
                row = dx * 18 + hc * 3 + c
                for hj in range(4):
                    ky = hc - hj
                    if 0 <= ky <= 2:
                        W1L[row, hj * 32:(hj + 1) * 32] = W1[:, c, ky, dx]
    W1L = np.broadcast_to(W1L, (B4, 64, 128)).astype(bf16)

    # conv1 bias per psum partition (hj,o): b1[o] + tn*tw[o]  -> [128, B4]
    BIAS = np.zeros((128, B4), np.float32)
    for b in range(B4):
        BIAS[:, b] = np.tile(b1 + tn[b] * tw, 4)

    # conv2 stationary variants [var(3) x dx(3)] each [128, 32]
    L2 = np.zeros((3, 3, 128, 32), np.float32)
    for dxi in range(3):
        for op in range(3):
            for jp in (1, 2):
                m = op * 2 + (jp - 1)
                for dy in (-1, 0, 1):
                    hj = jp + dy
                    L2[:, dxi, hj * 32:hj * 32 + HID, m] = W2[op, :, dy + 1, dxi]
        for op in range(3):           # var1: s=0, extra h=0 outputs at cols 6..8
            for dy in (0, 1):
                L2[1, dxi, dy * 32:dy * 32 + HID, 6 + op] = W2[op, :, dy + 1, dxi]
        for op in range(3):           # var2: s=126, extra h=255 outputs
            for dy in (-1, 0):
                hj = 3 + dy
                L2[2, dxi, hj * 32:hj * 32 + HID, 6 + op] = W2[op, :, dy + 1, dxi]
    L2 = L2.reshape(9, 128, 32).astype(bf16)

    # x_loss [B4, 128, 32, 256]: rows 32*sub + m hold x - b2
    xl = np.zeros((B4, 128, 32, 256), np.float32)
    for S in range(32):
        for sub in range(4):
            s = 4 * S + sub
            if s >= NS:
                continue
            for op in range(3):
                for jp in (1, 2):
                    m = op * 2 + (jp - 1)
                    xl[:, 32 * sub + m, S, :] = xs[:, op, 2 * s + jp, :] - b2[op]
            if s == 0:
                for op in range(3):
                    xl[:, 6 + op, S, :] = xs[:, op, 0, :] - b2[op]
            if s == 126:
                for op in range(3):
                    xl[:, 64 + 6 + op, S, :] = xs[:, op, 255, :] - b2[op]
    XL = xl.astype(bf16)

    X = xs.astype(bf16)
    return {"X": X, "MT": Mt, "W1L": W1L, "BIAS": BIAS,
            "L2": L2, "XL": XL}


def _build_module():
    import concourse.bacc as bacc
    import concourse.tile as tile
    from concourse import mybir
    from concourse.ap import AP

    BF = mybir.dt.bfloat16
    F32 = mybir.dt.float32
    RELU = mybir.ActivationFunctionType.Relu
    SQUARE = mybir.ActivationFunctionType.Square

    nc = bacc.Bacc("TRN2", target_bir_lowering=False, debug=False,
                   num_devices=NCORES)
    dX = nc.dram_tensor("X", [B4, C, H, W], BF, kind="ExternalInput").ap()
    dMT = nc.dram_tensor("MT", [B4, 256, 256], BF, kind="ExternalInput").ap()
    dW1L = nc.dram_tensor("W1L", [B4, 64, 128], BF, kind="ExternalInput").ap()
    dBIAS = nc.dram_tensor("BIAS", [128, B4], F32, kind="ExternalInput").ap()
    dL2 = nc.dram_tensor("L2", [9, 128, 32], BF, kind="ExternalInput").ap()
    dXL = nc.dram_tensor("XL", [B4, 128, 32, 256], BF, kind="ExternalInput").ap()
    dACC = nc.dram_tensor("ACC", [128, 32], F32, kind="ExternalOutput").ap()
    # internal DRAM staging for the blurred image, h- and w-padded:
    # layout [h_pad(258), c(3), w_pad(258)]
    dZ2 = [nc.dram_tensor(f"ZSTAGE{i}", [258, C, PW], BF).ap()
           for i in range(2)]

    with tile.TileContext(nc) as tc:
        from contextlib import ExitStack
        ctx = ExitStack()
        persist = ctx.enter_context(tc.tile_pool(name="persist", bufs=1))
        io = ctx.enter_context(tc.tile_pool(name="io", bufs=2))
        hpool = ctx.enter_context(tc.tile_pool(name="hpool", bufs=2))
        dpool = ctx.enter_context(tc.tile_pool(name="dpool", bufs=3))
        psA = ctx.enter_context(tc.tile_pool(name="psA", bufs=2, space="PSUM"))
        ps1 = ctx.enter_context(tc.tile_pool(name="ps1", bufs=2, space="PSUM"))
        ps2 = ctx.enter_context(tc.tile_pool(name="ps2", bufs=1, space="PSUM"))

        rpool = ctx.enter_context(tc.tile_pool(name="rpool", bufs=2))

        # persistent tiles
        acc = persist.tile([128, 32], F32, tag="acc")
        l2 = persist.tile([128, 9 * 32], BF, tag="l2")
        w1l = persist.tile([128, B4 * 128], BF, tag="w1l")
        bias = persist.tile([128, B4], F32, tag="bias")
        zrow = persist.tile([2, ZPITCH], BF, tag="zrow")

        # one-time init
        nc.gpsimd.memset(acc[:], 0.0)
        nc.gpsimd.memset(zrow[:], 0.0)
        # zero the h-pad rows (0 and 257) of both DRAM z staging buffers
        for i in range(2):
            nc.sync.dma_start(AP(dZ2[i].tensor, 0,
                                 [[257 * ZPITCH, 2], [1, ZPITCH]]), zrow[:])
        nc.sync.dma_start(l2[:], AP(dL2.tensor, 0,
                                    [[32, 128], [128 * 32, 9], [1, 32]]))
        # duplicate conv1 weights into both row-tile blocks (rows 0-63, 64-127)
        for blk in range(2):
            nc.sync.dma_start(w1l[64 * blk:64 * blk + 64, :],
                              AP(dW1L.tensor, 0,
                                 [[128, 64], [64 * 128, B4], [1, 128]]))
        nc.sync.dma_start(bias[:], dBIAS[:])
        RP = 64 * 256  # r1 free pitch per parity block (64 strip slots)

        for b in range(B4):
            # ---------------- load inputs for sample b ----------------
            mt = [io.tile([128, 256], BF, tag=f"mt{k}", name=f"mt{k}") for k in range(2)]
            for k in range(2):
                nc.gpsimd.dma_start(mt[k][:], dMT[b, 128 * k:128 * (k + 1), :])
            xc = [[io.tile([128, 256], BF, tag=f"xc{c}{k}", name=f"xc{c}{k}") for k in range(2)]
                  for c in range(C)]
            for c in range(C):
                for k in range(2):
                    nc.gpsimd.dma_start(xc[c][k][:],
                                        dX[b, c, 128 * k:128 * (k + 1), :])

            dZ = dZ2[b % 2]
            r1 = rpool.tile([128, 64 * 256], BF, tag="r1", name=f"r1_{b}")
            zn = [rpool.tile([128, ZPITCH], BF, tag=f"zn{k}", name=f"zn{k}_{b}")
                  for k in range(2)]
            at = [rpool.tile([128, C * 256], BF, tag=f"at{k}", name=f"at{k}_{b}")
                  for k in range(2)]
            # zero the w-pad columns of zn (cols c*258+0 / +257)
            for k in range(2):
                for colo in (0, 257):
                    nc.gpsimd.memset(AP(zn[k][:].tensor, zn[k][:].offset + colo,
                                        [[ZPITCH, 128], [258, C], [1, 1]]), 0.0)

            # ---------------- blur pass A: AT = X^T @ Mt ----------------
            for c in range(C):
                for wk in range(2):
                    pa = psA.tile([128, 256], F32, tag="pab")
                    for hk in range(2):
                        nc.tensor.matmul(pa[:],
                                         xc[c][hk][:, 128 * wk:128 * (wk + 1)],
                                         mt[hk][:], start=(hk == 0), stop=(hk == 1))
                    nc.vector.tensor_copy(at[wk][:, 256 * c:256 * (c + 1)], pa[:])

            # ---------------- blur pass B: z chunks (h' in [0,128),[128,256)) ----
            for c in range(C):
                for mk in range(2):
                    pb = psA.tile([128, 256], F32, tag="pab")
                    for wk in range(2):
                        nc.tensor.matmul(pb[:],
                                         at[wk][:, 256 * c + 128 * mk:
                                                256 * c + 128 * mk + 128],
                                         mt[wk][:], start=(wk == 0), stop=(wk == 1))
                    nc.vector.tensor_copy(zn[mk][:, PW * c + 1:PW * c + 257], pb[:])

            # stage z to DRAM: zn[k] [h-part, (c,w)] -> dZ rows 1+128k..128+128k
            for k in range(2):
                nc.scalar.dma_start(
                    AP(dZ.tensor, (1 + 128 * k) * ZPITCH, [[ZPITCH, 128], [1, ZPITCH]]),
                    zn[k][:])

            # ---------------- R1 gather: 6 bulk DMAs from DRAM ----------------
            # row block p=s&1 (partitions 64p+dxi*18+..), free slot s2=s>>1
            # R1[(p,dxi,hc,c), (s2,w)] = z[c, 2s-1+hc, w+dxi-1] (padded idx)
            for par in range(2):
                n2 = 64 - par  # 64 even strips (0..126), 63 odd (1..125)
                for dxi in range(3):
                    in_ap = AP(dZ.tensor, 2 * par * ZPITCH + dxi,
                               [[258, 18], [4 * ZPITCH, n2], [1, 256]])
                    out_ap = AP(r1[:].tensor,
                                r1[:].offset + (64 * par + dxi * 18) * RP,
                                [[RP, 18], [256, n2], [1, 256]])
                    (nc.sync if dxi != 1 else nc.scalar).dma_start(out_ap, in_ap)

            # ---------------- banded conv1 -> H -> conv2 -> loss ----------------
            for band in range(4):
                sband = 32 * band
                hbuf = hpool.tile([128, 32 * PW], BF, tag="H")
                # zero the w-pad columns (cheap: 2x 32 elems/partition)
                for colo in (0, 257):
                    zp = AP(hbuf[:].tensor, hbuf[:].offset + colo,
                            [[32 * PW, 128], [PW, 32], [1, 1]])
                    nc.gpsimd.memset(zp, 0.0)

                # conv1: quads of strips share one 4-bank psum tile
                for q in range(8):
                    sq = sband + 4 * q
                    if sq >= NS:
                        break
                    po = ps1.tile([128, 1024], F32, tag="po")
                    nq = min(4, NS - sq)
                    for par in range(2):
                        sp = [sq + i for i in range(nq) if (sq + i) & 1 == par]
                        if not sp:
                            continue
                        s2 = sp[0] >> 1
                        npar = len(sp)
                        nc.tensor.matmul(po[:, 512 * par:512 * par + 256 * npar],
                                         w1l[64 * par:64 * par + 54,
                                             128 * b:128 * (b + 1)],
                                         r1[64 * par:64 * par + 54,
                                            256 * s2:256 * (s2 + npar)],
                                         start=True, stop=True)
                        # relu+bias evac into H (strip segments sp), on ACT
                        lo = (sp[0] - sband) * PW
                        out_ap = AP(hbuf[:].tensor, hbuf[:].offset + lo + 1,
                                    [[32 * PW, 128], [2 * PW, npar], [1, 256]])
                        in_ap = AP(po[:].tensor, po[:].offset + 512 * par,
                                   [[1024, 128], [256, npar], [1, 256]])
                        nc.scalar.activation(out_ap, in_ap, RELU,
                                             bias=bias[:, b:b + 1])

                # conv2 + loss per S-quad (4 S-groups = 16 strips)
                xlb = dpool.tile([128, 2048], BF, tag="xl")
                nc.gpsimd.dma_start(
                    xlb[:], AP(dXL.tensor, dXL[b].offset + band * 8 * 256,
                               [[32 * 256, 128], [256, 8], [1, 256]]))
                for half in range(2):
                    p2 = ps2.tile([128, 1024], F32, tag="p2")
                    for pair in range(2):
                        S0 = 8 * band + 4 * half + 2 * pair
                        for sub in range(4):
                            strips = [4 * (S0 + j) + sub for j in range(2)]
                            strips = [s for s in strips if s < NS]
                            for s in (4 * S0 + sub, 4 * (S0 + 1) + sub):
                                if s >= NS:
                                    Sk = (s // 4) - (8 * band + 4 * half)
                                    nc.vector.memset(
                                        p2[32 * sub:32 * (sub + 1),
                                           256 * Sk:256 * (Sk + 1)], 0.0)
                            if not strips:
                                continue
                            plain = all(s != 0 and s != 126 for s in strips)
                            co = 512 * pair
                            if plain and len(strips) == 2:
                                sl = (strips[0] - sband) * PW
                                for dxi in range(3):
                                    rhs = AP(hbuf[:].tensor,
                                             hbuf[:].offset + sl + dxi,
                                             [[32 * PW, 128], [4 * PW, 2],
                                              [1, 256]])
                                    nc.tensor.matmul(
                                        p2[32 * sub:32 * (sub + 1),
                                           co:co + 512],
                                        l2[:, dxi * 32:(dxi + 1) * 32],
                                        rhs, start=(dxi == 0), stop=(dxi == 2),
                                        tile_position=(0, 32 * sub))
                            else:
                                for s in strips:
                                    Sk = (s // 4) - (8 * band + 4 * half)
                                    var = 1 if s == 0 else (2 if s == 126 else 0)
                                    sl = (s - sband) * PW
                                    for dxi in range(3):
                                        nc.tensor.matmul(
                                            p2[32 * sub:32 * (sub + 1),
                                               256 * Sk:256 * (Sk + 1)],
                                            l2[:, (var * 3 + dxi) * 32:
                                                  (var * 3 + dxi + 1) * 32],
                                            hbuf[:, sl + dxi:sl + dxi + 256],
                                            start=(dxi == 0), stop=(dxi == 2),
                                            tile_position=(0, 32 * sub))
                    # d = psum - x ; acc += d^2
                    dsb = dpool.tile([128, 1024], BF, tag="d")
                    nc.vector.tensor_sub(dsb[:], p2[:],
                                         xlb[:, 1024 * half:1024 * (half + 1)])
                    jsb = dpool.tile([128, 1024], BF, tag="j")
                    col = b * 8 + band * 2 + half
                    nc.scalar.activation(jsb[:], dsb[:], SQUARE,
                                         accum_out=acc[:, col:col + 1])

        nc.sync.dma_start(dACC[:], acc[:])
        ctx.close()

    nc.compile()
    return nc


def kernel(x, t, W1, b1, tw, W2, b2, sigma_schedule):
    from concourse.bass_utils import run_bass_kernel_spmd

    if "nc" not in _cached:
        _cached["nc"] = _build_module()
    nc = _cached["nc"]

    in_maps = []
    for core in range(NCORES):
        shard = list(range(core * B4, (core + 1) * B4))
        in_maps.append(_host_prep(x, t, W1, b1, tw, W2, b2, sigma_schedule,
                                  shard))
    res = run_bass_kernel_spmd(nc, in_maps, list(range(NCORES)))
    total = 0.0
    for r in res.results:
        total += float(r["ACC"].astype(np.float64).sum())
    out = np.float32(total / (B * C * H * W))
    return np.asarray(out)


if __name__ == "__main__":
    sys.path.insert(0, os.path.dirname(os.path.abspath(__file__)))
    import reference
    inputs = {k: np.asarray(v) for k, v in reference.setup_inputs().items()}
    expected = float(reference.reference(**inputs))
    got = kernel(**inputs)
    rel = abs(float(got) - expected) / abs(expected)
    print("expected", expected, "got", float(got), "rel", rel)
